# revision 16
# baseline (speedup 1.0000x reference)
"""ButterflyMlp Trainium2 kernel (banded-permutation, parity-PSUM version).

Reference computation (B=65536):
    h1 = relu(x @ (W1*m1).T + b1)          # [B, 784]
    h2 = relu(h1 @ (W2*m2).T + b2)         # [B, 128]
    logits = h2 @ (W3*m3).T + b3           # [B, 10]
    out = log_softmax(logits, axis=1)

Strategy: pure data parallel over 8 NeuronCores (batch sharded 8192/core,
masked weights replicated), fp8e4m3 with fp32 PSUM accumulation.

Every nonzero offset d = j - o of the butterfly mask m1 satisfies
(d mod 156) in [-10, 10]: sorting both the 784 input features and the 784
h1 outputs by (index mod 156) turns W1*m1 into a banded matrix.  The
outputs are split into 7 tiles of 112 (padded to 128 lanes) and the
features laid into 8 SBUF "slots" of 128 rows such that tile t's window
is covered by slots (t, t+1).  Layer 1 is then 7 single DoubleRow fp8
matmuls (K = 256) per batch block instead of a dense 784-row contraction.

The steady state is bound by PSUM evacuation: only the Vector and
Scalar engines can read PSUM, both at 1 elem/cycle/lane for fp32
sources, plus ~120-220 cycles per op and ~120 ns per semaphore wait.
Matmuls stay at 512 columns (256-col DoubleRow matmuls double the
LDWEIGHTS traffic since DR disables the fast weight load: measured 2.6x
slower) and PSUM tiles stay separate per evacuation op (the framework
tracks hazards at tile granularity: one shared 7-bank tile serializes
the whole kernel: also measured 2.6x slower).  Layer-1 PSUM is grouped
2+2+3 banks so seven tile evacuations become three wide ops: DVE takes
d01 + d23 (1024 cols each), ACT takes d456 (1536 cols) followed by the
previous block's layer-2 relu, balancing the engines' measured rates
(ACT ~1.09 ns/col, DVE ~1.19 ns/col) including per-op and semaphore
overheads.

Layer 2 is 4 DoubleRow matmuls per block (the old 7th plain fp8 matmul,
which costs 2x a DoubleRow on HW, is folded away by repeating the
(t5, t6) slot pair with the t5 half of the weights zeroed).  It shares
the 8th PSUM bank with layer 3's logits, emitted one block late so its
matmuls never head the in-order PE queue waiting on h1 evacuations.
Layer 3 + log_softmax run per 4-block group: logits matmuls write the
spare region of the L2 bank, z is scaled out by the Vector engine, Exp
runs on the Scalar engine directly from PSUM with the 1/SW^2 scale
folded into the activation's affine stage, and ln plus the final
subtraction are deferred to a single epilogue so the scalar activation
table loads only twice.

The masked weights are pre-scaled by 32 (h1 stored at scale 32, h2 at
1024); the scales fold into the relu / softmax stages.  End-to-end max
relative error vs the fp32 reference is ~3e-4.
"""

import numpy as np
import ml_dtypes

import concourse.bass as bass
import concourse.mybir as mybir
import concourse.tile as tile
from concourse import bacc
from concourse.bass_utils import run_bass_kernel_spmd

BF16 = ml_dtypes.bfloat16
FP8 = ml_dtypes.float8_e4m3
F32 = np.float32

N_CORES = 8
B = 65536
S = B // N_CORES          # batch rows per core
IN_F = 784
PERIOD = 156              # stripe period of the 784x784 butterfly mask
NT = 7                    # h1 output tiles (112 real outputs each)
TR = 112                  # real outputs per tile
NS = 8                    # x feature slots (chain: tile t reads slots t, t+1)
NP2 = 4                   # layer-2 DoubleRow pairs (4th = (t5, t6), t5 half zeroed)
H2 = 128
NCLS = 10
NSMX = 16                 # layer-3 batch tiles per softmax group
NGRP = 4                  # softmax groups per core
NBLK = S // 512           # 512-column batch blocks per core (16)
HB = 256                  # half-block columns (parity granularity)

SW = 32.0                 # fp8 weight pre-scale; h1 at scale SW, h2 at SW*SW



WINDOW, STRIPES, STEP = 10, 5, 3

_CACHE = {}


def _butterfly_mask(out_f, in_f, window=WINDOW, stripes=STRIPES, step=STEP):
    i = np.arange(out_f)[:, None]
    j = np.arange(in_f)[None, :]
    jc = (i * in_f) // out_f
    band = np.abs(j - jc) <= window
    period = max(in_f // stripes, 1)
    stripe = ((j - jc) % period) < step
    return (band | stripe).astype(np.float32)


def _build_layout():
    """o_tiles: 7 lists of 112 output ids (residue-sorted); slots: 8 lists
    of 128 feature ids (-1 = pad) covering tile t's window in slots t,t+1."""
    o = np.arange(IN_F)
    out_perm = o[np.lexsort((o // PERIOD, o % PERIOD))]
    o_tiles = [out_perm[TR * t: TR * (t + 1)] for t in range(NT)]

    wins = []
    for t in range(NT):
        r = np.sort(np.unique(o_tiles[t] % PERIOD))
        wc = np.array([(c % PERIOD) for c in range(r[0] - 10, r[-1] + 11)])
        Wt = np.arange(IN_F)[np.isin(np.arange(IN_F) % PERIOD, wc)]
        wins.append(Wt[np.lexsort((Wt // PERIOD, Wt % PERIOD))])

    slots = [None] * NS
    w1set = set(wins[1].tolist())
    first = [j for j in wins[0] if j not in w1set]
    slots[0] = np.array(first + [-1] * (128 - len(first)))
    for t in range(NT):
        in_prev = set(slots[t][slots[t] >= 0].tolist())
        rest = [j for j in wins[t] if j not in in_prev]
        assert len(rest) <= 128
        slots[t + 1] = np.array(rest + [-1] * (128 - len(rest)))
    return o_tiles, slots


def _build_nc(ZB):
    nc = bacc.Bacc("TRN2", target_bir_lowering=False, debug=False, num_devices=N_CORES)

    # x block-major: [part, block, slot, col] so a group DMA moves 16KB
    # contiguous per partition.  Weights/bias host-packed per the layout.
    xq = nc.dram_tensor("xq", [128, NBLK, NS, 512], mybir.dt.float8e4, kind="ExternalInput")
    w1q = nc.dram_tensor("w1q", [128, NT * 2 * 128], mybir.dt.float8e4, kind="ExternalInput")
    w2q = nc.dram_tensor("w2q", [128, NP2 * 2 * H2], mybir.dt.float8e4, kind="ExternalInput")
    w3q = nc.dram_tensor("w3q", [H2, NCLS], mybir.dt.bfloat16, kind="ExternalInput")
    bias = nc.dram_tensor("bias", [128, NT + 1 + NCLS], mybir.dt.float32, kind="ExternalInput")
    out = nc.dram_tensor("out", [S, NCLS], mybir.dt.float32, kind="ExternalOutput")

    Relu = mybir.ActivationFunctionType.Relu
    Exp = mybir.ActivationFunctionType.Exp
    X = mybir.AxisListType.X
    DR = mybir.MatmulPerfMode.DoubleRow
    ADD = mybir.AluOpType.add
    MAX = mybir.AluOpType.max
    MULT = mybir.AluOpType.mult

    with tile.TileContext(nc) as tc:
        with (
            tc.tile_pool(name="consts", bufs=1) as consts,
            tc.tile_pool(name="spool", bufs=3) as spool,
            tc.tile_pool(name="psum", bufs=1, space="PSUM") as psum,
        ):
            # PE warm-up: dummy matmuls during the initial DMA wait flip the
            # HAM clock gate toward full rate before the real matmuls arrive.
            warm = consts.tile([128, 512], mybir.dt.float8e4)
            nc.vector.memset(warm[:], 0.0)
            warm_ps = psum.tile([128, 512], mybir.dt.float32, tag="l2", bufs=1)
            for i in range(6):
                nc.tensor.matmul(
                    warm_ps[:],
                    warm[:, 0:128],
                    warm[:],
                    start=(i == 0),
                    stop=(i == 5),
                    skip_group_check=True,
                )

            # weights/x interleaved so the first output tiles' inputs land
            # quickly; remaining x streams in behind compute.
            w1r = w1q.rearrange("p (t s o) -> p t s o", t=NT, s=2)
            w1_sb = consts.tile([128, NT, 2, 128], mybir.dt.float8e4)
            xt_all = consts.tile([128, NBLK, NS, 512], mybir.dt.float8e4)
            # first-block inputs issued from multiple engines in parallel so
            # the transfers all start right after queue bring-up
            nc.sync.dma_start(xt_all[:, 0, 0:3], xq[:, 0, 0:3])
            nc.scalar.dma_start(w1_sb[:, 0:2], w1r[:, 0:2])
            nc.gpsimd.dma_start(w1_sb[:, 2:7], w1r[:, 2:7])
            nc.sync.dma_start(xt_all[:, 0, 3:8], xq[:, 0, 3:8])
            nc.gpsimd.dma_start(xt_all[:, 1], xq[:, 1])

            w2_sb = consts.tile([128, NP2, 2, H2], mybir.dt.float8e4)
            nc.sync.dma_start(w2_sb[:], w2q.rearrange("p (k s o) -> p k s o", k=NP2, s=2))
            w3_sb = consts.tile([128, NCLS], mybir.dt.bfloat16)
            nc.sync.dma_start(w3_sb[:], w3q[:, :])
            bias_sb = consts.tile([128, NT + 1 + NCLS], mybir.dt.float32)
            nc.sync.dma_start(bias_sb[:], bias[:, :])
            b1_sb = bias_sb[:, 0:NT]
            b2_sb = bias_sb[:, NT : NT + 1]
            b3_sb = bias_sb[:, NT + 1 :]

            # remaining x in 2-block chunks alternating between the sync and
            # gpsimd rings so the per-queue serial transfer time halves and
            # each chunk's completion unblocks compute sooner
            nc.sync.dma_start(xt_all[:, 2], xq[:, 2])
            nc.sync.dma_start(xt_all[:, 3], xq[:, 3])
            for c in range(2, NBLK // 2):
                eng = nc.sync if c % 2 == 0 else nc.gpsimd
                eng.dma_start(
                    xt_all[:, 2 * c : 2 * c + 2], xq[:, 2 * c : 2 * c + 2]
                )

            # persistent whole-shard activations + deferred-softmax state
            h1_all = consts.tile([128, NT, S], mybir.dt.float8e4)
            h2_all = consts.tile([128, S], mybir.dt.bfloat16)
            z_all = consts.tile([128, NGRP, NSMX, NCLS], mybir.dt.float32)
            se_all = consts.tile([128, NGRP, NSMX], mybir.dt.float32)

            def l2_mms(nb_p):
                # 4 DoubleRow matmuls, pairs (0,1),(2,3),(4,5),(5,6); the
                # last pair's t5 half has zeroed weights so all four run at
                # the 0.5 cyc/row fp8 DoubleRow rate.
                ns_p = slice(nb_p * 512, (nb_p + 1) * 512)
                ps_l2 = psum.tile([128, 512], mybir.dt.float32, tag="l2", bufs=1)
                pairs = [(0, 2), (2, 4), (4, 6), (5, 7)]
                for k, (a, b) in enumerate(pairs):
                    nc.tensor.matmul(
                        ps_l2[:],
                        w2_sb[:, k],
                        h1_all[:, a:b, ns_p],
                        start=(k == 0),
                        stop=(k == NP2 - 1),
                        perf_mode=DR,
                    )
                return ps_l2

            def l2_evac(ps_l2, nb_p, eng="act"):
                # psum = SW^2 * (h1 @ W2m.T); h2 stored at scale SW^2.
                ns_p = slice(nb_p * 512, (nb_p + 1) * 512)
                if eng == "act":
                    if ZB:
                        nc.scalar.activation(h2_all[:, ns_p], ps_l2[:], Relu)
                    else:
                        nc.scalar.activation(
                            h2_all[:, ns_p], ps_l2[:], Relu, bias=b2_sb[:, 0:1]
                        )
                else:
                    nc.vector.tensor_scalar(
                        h2_all[:, ns_p], ps_l2[:], b2_sb[:, 0:1], 0.0, ADD, MAX
                    )

            def l3_mms(g, bts):
                # the logits live in the d01 tag's bank pair (free mid-block
                # once d01's evac completes) so they never serialize against
                # the next block's layer-2 matmuls in the single l2 bank
                nbt = len(bts)
                ps_l3 = psum.tile([128, nbt, NCLS], mybir.dt.float32, tag="d01")
                for i, bt in enumerate(bts):
                    bt_abs = g * NSMX + bt
                    nc.tensor.matmul(
                        ps_l3[:, i, :],
                        h2_all[:, bt_abs * 128 : (bt_abs + 1) * 128],
                        w3_sb[:, :],
                        start=(i == 0),
                        stop=(i == nbt - 1),
                        skip_group_check=True,
                    )
                return ps_l3

            def l3_zs(ps_l3, g, bts):
                # z = psum / SW^2 (+ b3): Vector reads PSUM once
                nbt = len(bts)
                zs = z_all[:, g, bts[0] : bts[0] + nbt]
                if ZB:
                    nc.vector.tensor_scalar(zs, ps_l3[:], 1.0 / (SW * SW), 0.0, MULT, ADD)
                else:
                    nc.vector.scalar_tensor_tensor(
                        zs, ps_l3[:], 1.0 / (SW * SW),
                        b3_sb[:, None, :].to_broadcast((128, nbt, NCLS)),
                        MULT, ADD,
                    )

            def l3_exp(ps_l3, g, bts):
                # e = exp(z): Scalar reads PSUM directly with the 1/SW^2
                # scale folded into the activation's affine stage (ZB), so
                # the exp does not wait on the Vector engine's z scale-out.
                nbt = len(bts)
                e = spool.tile([128, nbt, NCLS], mybir.dt.float32, tag="e")
                if ZB:
                    nc.scalar.activation(e[:], ps_l3[:], Exp, bias=0.0, scale=1.0 / (SW * SW))
                else:
                    zs = z_all[:, g, bts[0] : bts[0] + nbt]
                    nc.scalar.activation(e[:], zs, Exp)
                return e

            def l3_reduce(g, bts, e):
                nc.vector.reduce_sum(se_all[:, g, bts[0] : bts[0] + len(bts)], e[:], axis=X)

            def l3_stage(nb_p):
                # Groups 0-2 are processed 16 batch-tiles at once when their
                # last block's h2 lands; the last group goes in 8/4/4 stages
                # so the end-of-kernel dependency chain is short.
                if nb_p < 4 * (NGRP - 1):
                    if nb_p % 4 != 3:
                        return None
                    return nb_p // 4, range(NSMX)
                elif nb_p == 4 * (NGRP - 1):
                    return None
                elif nb_p == 4 * (NGRP - 1) + 1:
                    return NGRP - 1, range(0, 8)
                else:
                    return NGRP - 1, range((nb_p % 4) * 4, (nb_p % 4) * 4 + 4)

            def l1_evac(dtile, t0, t1, nb, eng):
                # evacuate tiles [t0, t1) of block nb from `dtile` in one op
                dst = h1_all[:, t0:t1, nb * 512 : (nb + 1) * 512]
                src = dtile[:, 0 : t1 - t0]
                if ZB:
                    if eng == "act":
                        nc.scalar.activation(dst, src, Relu, bias=0.0, scale=1.0)
                    else:
                        nc.vector.tensor_scalar(dst, src, 0.0, 0.0, ADD, MAX)
                else:
                    for t in range(t0, t1):
                        dst_t = h1_all[:, t, nb * 512 : (nb + 1) * 512]
                        src_t = dtile[:, t - t0]
                        if eng == "act":
                            nc.scalar.activation(
                                dst_t, src_t, Relu, bias=b1_sb[:, t : t + 1], scale=1.0
                            )
                        else:
                            nc.vector.tensor_scalar(
                                dst_t, src_t, b1_sb[:, t : t + 1], 0.0, ADD, MAX
                            )

            def epilogue():
                # ln of all rowsums, one wide subtraction, one output DMA.
                # ln(se) without the 1.28us Ln table load: y0 from the
                # float exponent field (linear-mantissa log, err <= 0.06),
                # refined via the already-resident Exp table:
                # r = se*exp(-y0) = 1+eps, ln(se) = y0 + eps - eps^2/2
                # (cubic error <= 8e-5, inside the error budget).
                C1 = float(np.log(2.0) / (1 << 23))
                C2 = float(127.0 * np.log(2.0))
                u = spool.tile([128, NGRP, NSMX], mybir.dt.float32, tag="lu")
                nc.vector.tensor_copy(u[:], se_all[:].bitcast(mybir.dt.uint32))
                m = spool.tile([128, NGRP, NSMX], mybir.dt.float32, tag="lm")
                nc.vector.tensor_scalar(m[:], u[:], -C1, C2, MULT, ADD)
                e0 = spool.tile([128, NGRP, NSMX], mybir.dt.float32, tag="le")
                nc.scalar.activation(e0[:], m[:], Exp)
                r = spool.tile([128, NGRP, NSMX], mybir.dt.float32, tag="lr")
                nc.vector.scalar_tensor_tensor(r[:], se_all[:], 1.0, e0[:], MULT, MULT)
                h = spool.tile([128, NGRP, NSMX], mybir.dt.float32, tag="lh")
                nc.vector.tensor_scalar(h[:], r[:], -0.5, 1.5, MULT, ADD)
                p = spool.tile([128, NGRP, NSMX], mybir.dt.float32, tag="lp")
                nc.vector.scalar_tensor_tensor(p[:], r[:], -1.0, h[:], ADD, MULT)
                lse = spool.tile([128, NGRP, NSMX], mybir.dt.float32, tag="lse")
                nc.vector.tensor_sub(lse[:], p[:], m[:])
                # two halves so the first DMA's transfer overlaps the
                # second subtraction; batch is host-permuted so partition p
                # owns 64 globally consecutive output rows (1280B runs per
                # half)
                og = spool.tile([128, NGRP, NSMX, NCLS], mybir.dt.float32, tag="og")
                outr = out[:, :].rearrange("(p g bt) c -> p g bt c", p=128, g=NGRP)
                nc.vector.tensor_sub(
                    og[:, 0:2],
                    z_all[:, 0:2],
                    lse[:, 0:2, :, None].to_broadcast((128, 2, NSMX, NCLS)),
                )
                nc.sync.dma_start(outr[:, 0:2], og[:, 0:2])
                nc.vector.tensor_sub(
                    og[:, 2:4],
                    z_all[:, 2:4],
                    lse[:, 2:4, :, None].to_broadcast((128, 2, NSMX, NCLS)),
                )
                nc.scalar.dma_start(outr[:, 2:4], og[:, 2:4])

            def stage_pieces(nb_p):
                st = l3_stage(nb_p)
                if st is None:
                    return None
                g, bts = st
                ps_l3 = l3_mms(g, bts)
                l3_zs(ps_l3, g, bts)
                e = l3_exp(ps_l3, g, bts)
                l3_reduce(g, bts, e)
                return True

            # ---- main loop ----------------------------------------------
            # Per block: 7 DoubleRow L1 matmuls into three tag-rotated PSUM
            # tiles (2+2+3 banks), the previous block's L2 matmuls, then the
            # evacuations: DVE d01 + d23 (1024 cols each), ACT d456 (1536)
            # followed by the previous block's L2 relu.  Tag rotation gives
            # per-tile bank-reuse dependencies (tile granularity is what the
            # framework tracks) so the PE's next block overlaps the current
            # block's evacuation.
            pending = None
            for nb in range(NBLK):
                d01 = psum.tile([128, 2, 512], mybir.dt.float32, tag="d01")
                d23 = psum.tile([128, 2, 512], mybir.dt.float32, tag="d23")
                d456 = psum.tile([128, 3, 512], mybir.dt.float32, tag="d456")
                slices = [d01[:, 0], d01[:, 1], d23[:, 0], d23[:, 1],
                          d456[:, 0], d456[:, 1], d456[:, 2]]
                for t in range(NT):
                    nc.tensor.matmul(
                        slices[t],
                        w1_sb[:, t],
                        xt_all[:, nb, t : t + 2, :],
                        start=True,
                        stop=True,
                        perf_mode=DR,
                        skip_group_check=True,
                    )
                ps_l2 = None
                if pending is not None:
                    ps_l2 = l2_mms(pending)
                if nb < NBLK - 1:
                    # DVE queue: d01 then d23
                    l1_evac(d01, 0, 2, nb, "vec")
                    l1_evac(d23, 2, 4, nb, "vec")
                    # ACT queue: d456 first (ready right after the t6
                    # matmul), then the previous block's h2
                    l1_evac(d456, 4, 7, nb, "act")
                else:
                    # last block: split across both engines so h1 completes
                    # quickly and the end-of-kernel chain stays short
                    l1_evac(d01, 0, 2, nb, "vec")
                    l1_evac(d23, 2, 4, nb, "act")
                    nc.vector.tensor_scalar(
                        h1_all[:, 6, nb * 512 : (nb + 1) * 512],
                        d456[:, 2], b1_sb[:, 6:7] if not ZB else 0.0, 0.0, ADD, MAX,
                    )
                    if ZB:
                        nc.scalar.activation(
                            h1_all[:, 4:6, nb * 512 : (nb + 1) * 512],
                            d456[:, 0:2], Relu, bias=0.0, scale=1.0,
                        )
                    else:
                        for t in (4, 5):
                            nc.scalar.activation(
                                h1_all[:, t, nb * 512 : (nb + 1) * 512],
                                d456[:, t - 4], Relu,
                                bias=b1_sb[:, t : t + 1], scale=1.0,
                            )
                if ps_l2 is not None:
                    l2_evac(ps_l2, pending, "act")
                    stage_pieces(pending)
                pending = nb

            # flush: final block's layer 2 + last softmax stage + epilogue
            ps_l2 = l2_mms(pending)
            l2_evac(ps_l2, pending, "act")
            stage_pieces(pending)
            epilogue()

    return nc


def _shard_perm():
    """Shard position g*2048 + bt*128 + p processes original row
    p*64 + g*16 + bt, so each partition owns 64 consecutive output rows
    (one contiguous 2560B DMA run per partition)."""
    pos = np.arange(S)
    g, r = np.divmod(pos, NSMX * 128)
    bt, p = np.divmod(r, 128)
    return p * (NGRP * NSMX) + g * NSMX + bt


def _prep_inputs(x, W1, b1, W2, b2, W3, b3):
    m1 = _butterfly_mask(IN_F, IN_F)
    m2 = _butterfly_mask(H2, IN_F)
    m3 = _butterfly_mask(NCLS, H2)
    o_tiles, slots = _build_layout()

    w1t = (np.asarray(W1, F32) * m1).T * SW     # [j, o]
    w2t = (np.asarray(W2, F32) * m2).T * SW     # [j, o2]

    # w1 layout [p, t, s, o]: weight of feature slots[t+s][p] for output
    # o_tiles[t][o]; zero at pads.
    w1l = np.zeros((128, NT, 2, 128), dtype=F32)
    for t in range(NT):
        cols = o_tiles[t]
        for s in range(2):
            rows = slots[t + s]
            valid = rows >= 0
            w1l[valid, t, s, :TR] = w1t[np.ix_(rows[valid], cols)]
    w1l = np.ascontiguousarray(w1l.reshape(128, NT * 2 * 128)).astype(FP8)

    # w2 layout [p, k, s, o2]: DoubleRow pair k half s holds the weight of
    # h1 feature o_tiles[pair[k][s]][p]; the 4th pair is (t5, t6) with the
    # t5 half zeroed (t5 is already contracted by pair 2).
    pairs = [(0, 1), (2, 3), (4, 5), (None, 6)]
    w2l = np.zeros((128, NP2, 2, H2), dtype=F32)
    for k, (a, b) in enumerate(pairs):
        if a is not None:
            w2l[:TR, k, 0, :] = w2t[o_tiles[a], :]
        if b is not None:
            w2l[:TR, k, 1, :] = w2t[o_tiles[b], :]
    w2l = np.ascontiguousarray(w2l.reshape(128, NP2 * 2 * H2)).astype(FP8)

    w3l = ((np.asarray(W3, F32) * m3).T).astype(BF16).copy()

    # bias pack [128, 7 + 1 + 10] f32: b1 per tile (scaled by SW), b2
    # scaled by SW^2, b3 broadcast.
    biasp = np.zeros((128, NT + 1 + NCLS), F32)
    b1f = np.asarray(b1, F32) * SW
    for t in range(NT):
        biasp[:TR, t] = b1f[o_tiles[t]]
    biasp[:, NT] = np.asarray(b2, F32) * (SW * SW)
    biasp[:, NT + 1 :] = np.asarray(b3, F32)[None, :]
    biasp = np.ascontiguousarray(biasp)

    # x: [B, 784] -> fp8 slot layout, batch permuted within each group,
    # block-major per core: xq[p, blk, slot, col]
    perm = _shard_perm()
    full_perm = np.concatenate([c * S + perm for c in range(N_CORES)])
    xT = np.asarray(x, F32).T.astype(FP8)[:, full_perm]    # [784, B]
    xs = np.zeros((NS, 128, B), dtype=FP8)
    for s in range(NS):
        rows = slots[s]
        valid = rows >= 0
        xs[s, valid] = xT[rows[valid]]

    in_maps = []
    for c in range(N_CORES):
        xc = xs[:, :, c * S : (c + 1) * S].reshape(NS, 128, NBLK, 512)
        xc = np.ascontiguousarray(xc.transpose(1, 2, 0, 3))   # [p, blk, s, col]
        in_maps.append(
            {
                "xq": xc,
                "w1q": w1l,
                "w2q": w2l,
                "w3q": w3l,
                "bias": biasp,
            }
        )
    return in_maps


def _run(inputs, trace=False, **run_kwargs):
    zb = bool(
        np.all(np.asarray(inputs["b1"]) == 0)
        and np.all(np.asarray(inputs["b2"]) == 0)
        and np.all(np.asarray(inputs["b3"]) == 0)
    )
    key = f"nc{zb}"
    if key not in _CACHE:
        nc = _build_nc(zb)
        nc.finalize()
        _CACHE[key] = nc
    nc = _CACHE[key]
    in_maps = _prep_inputs(**inputs)
    res = run_bass_kernel_spmd(
        nc,
        in_maps,
        core_ids=list(range(N_CORES)),
        trace=trace,
        **run_kwargs,
    )
    out = np.concatenate([r["out"] for r in res.results], axis=0)
    return out, res


def kernel(**inputs):
    out, _ = _run(inputs, trace=False)
    return out
